# revision 45
# baseline (speedup 1.0000x reference)
"""Causal self-attention Trainium2 kernel (B=4, T=2048, C=1024, H=16, D=64).

Sharding: 8 cores = 4 batches x 2 head-groups (8 heads each).
Each core computes qkv for its head group (column-split w_attn), full causal
attention for its 8 heads, and a partial c_proj (row-split w_proj).  The
partial outputs of the two head-group cores of each batch are summed on the
host (cheaper than a 2-rank on-device all-reduce), and b_proj is added there.

Device layout per core:
  x''      [2048, 1152]  = [x_b | ones | 0-pad]       (bias via matmul)
  w_qkv    [1152, 1536]  = [[w_attn cols for group]; [b_attn]; [0-pad]]
  w_o      [512, 1024]   = w_proj rows for group
  mask     [128, 128]    = upper-triangular (c >= r) multiplicative mask

Pipeline: PE-transpose x -> x^T ; qkv^T matmuls (f32r) giving q^T,k^T in
[D,T] layout and v in [T,D] layout (+ ones column for the softmax
denominator); S^T = (K^T)^T Q^T per 128-k x 512-q block with heads packed
2-per-PE (row groups 0:64 / 64:128); exp on ACT (scale=1/8) -> P^T with the
diagonal 128x128 block masked multiplicatively on DVE afterwards (columns
left of the diagonal are simply excluded from the restricted matmuls);
O^T = [V|1]^T P^T accumulated over k-tiles, row 64 is the softmax
denominator; normalize via DVE reciprocal + gpsimd partition broadcast,
writing y^T over the dead q^T storage; proj matmuls straight from that
y^T layout.
"""

import numpy as np

import concourse.bass as bass
import concourse.tile as tile
import concourse.mybir as mybir
from concourse import bacc, bass_utils
from concourse.masks import make_identity

F32 = mybir.dt.float32
F32R = mybir.dt.float32r
AF = mybir.ActivationFunctionType

B, T, C = 4, 2048, 1024
H = 16            # total heads
HG = 8            # heads per core (head group)
D = 64
CIN = 1152        # padded contraction: 1024 + 1 bias row + padding
NCI = CIN // 128  # 9 contraction tiles
NTT = T // 128    # 16 t tiles
NTB = 4           # t blocks of 512
NQ = 4            # q blocks of 512
NKJ = 16          # k tiles of 128

_NC_CACHE = {}


def build_kernel():
    nc = bacc.Bacc("TRN2", target_bir_lowering=False, debug=False)
    x_d = nc.dram_tensor("x", [T, CIN], F32, kind="ExternalInput").ap()
    wqkv_d = nc.dram_tensor("wqkv", [CIN, 1536], F32, kind="ExternalInput").ap()
    wo_d = nc.dram_tensor("wo", [512, C], F32, kind="ExternalInput").ap()
    mask_d = nc.dram_tensor("mask", [128, 128], F32, kind="ExternalInput").ap()
    out_d = nc.dram_tensor("out", [T, C], F32, kind="ExternalOutput").ap()

    with tile.TileContext(nc) as tc:
        with tc.tile_pool(name="persist", bufs=1) as persist:
            ident = persist.tile([128, 128], F32)
            make_identity(nc, ident[:])
            ones_f = persist.tile([128, 1], F32)
            nc.vector.memset(ones_f[:], 1.0)
            mask_sb = persist.tile([128, 128], F32R)
            nc.gpsimd.dma_start(mask_sb[:], mask_d)

            # q^T / k^T, head-pair-stacked: index r: 0-3 = q pairs, 4-7 = k
            # pairs.  After attention for pair hp, y^T overwrites the q half.
            qkT = persist.tile([128, 8, T], F32R)
            # v + ones column, per k-tile: [tt][h*65:(h+1)*65] = [V_h | 1]
            v_all = persist.tile([128, NKJ, 520], F32R)

            # ---------------- phase 1+2: transpose x and compute qkv ------
            with (
                tc.tile_pool(name="qkv_sb", bufs=2) as qkv_sb,
                tc.tile_pool(name="xnat", bufs=4) as xnat_pool,
                tc.tile_pool(name="wqk", bufs=2) as wqk_pool,
                tc.tile_pool(name="pst", bufs=2, space="PSUM") as pst_pool,
                tc.tile_pool(name="psv", bufs=2, space="PSUM") as psv_pool,
                tc.tile_pool(name="psq", bufs=3, space="PSUM") as psq_pool,
            ):
                # w_v stays resident: [128, 9, 512] f32r (cast during DMA)
                wv = persist.tile([128, NCI, 512], F32R)
                nc.gpsimd.dma_start(
                    wv[:],
                    wqkv_d[:, 1024:1536].rearrange("(ci p) co -> p ci co", p=128),
                )

                for tb in range(NTB):
                    xT = qkv_sb.tile([128, NCI, 512], F32R, tag="xT")
                    for tl in range(4):
                        tt = tb * 4 + tl
                        xn = xnat_pool.tile([128, CIN], F32)
                        nc.sync.dma_start(xn[:], x_d[tt * 128:(tt + 1) * 128, :])
                        for g0, glen in ((0, 4), (4, 4), (8, 1)):
                            pst = pst_pool.tile([128, 512], F32)
                            for cj in range(glen):
                                ci = g0 + cj
                                nc.tensor.transpose(
                                    pst[:, cj * 128:(cj + 1) * 128],
                                    xn[:, ci * 128:(ci + 1) * 128],
                                    ident[:],
                                )
                            nc.scalar.copy(
                                xT[:, g0:g0 + glen, tl * 128:(tl + 1) * 128],
                                pst[:, 0:glen * 128].rearrange(
                                    "p (g c) -> p g c", c=128),
                            )
                    # v for the 4 t-tiles of this block
                    for tl in range(4):
                        tt = tb * 4 + tl
                        psv = psv_pool.tile([128, 512], F32)
                        for ci in range(NCI):
                            nc.tensor.matmul(
                                psv[:], xT[:, ci, tl * 128:(tl + 1) * 128],
                                wv[:, ci, :],
                                start=(ci == 0), stop=(ci == NCI - 1),
                            )
                        vrow = v_all[:, tt, :].rearrange("p (h x) -> p h x", x=65)
                        nc.scalar.copy(
                            vrow[:, :, 0:64],
                            psv[:].rearrange("p (h d) -> p h d", d=64),
                        )
                        nc.vector.tensor_copy(
                            vrow[:, :, 64], ones_f[:].to_broadcast([128, 8]),
                        )
                    # q^T / k^T rows for this t block (two 512-col passes)
                    for half in range(2):
                        wqk = wqk_pool.tile([128, NCI, 512], F32R, tag="wqk")
                        nc.gpsimd.dma_start(
                            wqk[:],
                            wqkv_d[:, half * 512:(half + 1) * 512].rearrange(
                                "(ci p) co -> p ci co", p=128),
                        )
                        for rl in range(4):
                            r = half * 4 + rl
                            psq = psq_pool.tile([128, 512], F32, tag="psq")
                            for ci in range(NCI):
                                nc.tensor.matmul(
                                    psq[:], wqk[:, ci, rl * 128:(rl + 1) * 128],
                                    xT[:, ci, :],
                                    start=(ci == 0), stop=(ci == NCI - 1),
                                )
                            nc.scalar.copy(
                                qkT[:, r, tb * 512:(tb + 1) * 512], psq[:])

            # ---------------- phase 3: attention + projection -------------
            with (
                tc.tile_pool(name="attn_sb", bufs=7) as attn_sb,
                tc.tile_pool(name="norm_sb", bufs=8) as norm_sb,
                tc.tile_pool(name="wo_sb", bufs=1) as wo_pool,
                tc.tile_pool(name="pss", bufs=3, space="PSUM") as pss_pool,
                tc.tile_pool(name="pso", bufs=1, space="PSUM") as pso_pool,
                tc.tile_pool(name="io", bufs=4) as io_pool,
            ):
                wo_t = wo_pool.tile([128, 4, C], F32R)
                nc.gpsimd.dma_start(
                    wo_t[:], wo_d.rearrange("(cc p) co -> p cc co", p=128))

                for hp in range(4):
                    for i in range(NQ):
                        oa = pso_pool.tile([65, 512], F32, tag="oa")
                        ob = pso_pool.tile([65, 512], F32, tag="ob")
                        njt = 4 * i + 4
                        for j in range(njt):
                            m = j - 4 * i          # >=0 on diagonal k-tiles
                            off = max(m, 0) * 128  # first useful q column
                            # head A's block sits at psum cols [off, 512);
                            # head B's is shifted left to [512, 1024-off) so
                            # the written region is contiguous and one exp
                            # covers both heads.
                            ps = pss_pool.tile([128, 1024], F32, tag="ps")
                            nc.tensor.matmul(
                                ps[:, off:512],
                                qkT[0:64, 4 + hp, j * 128:(j + 1) * 128],
                                qkT[0:64, hp, i * 512 + off:(i + 1) * 512],
                                start=True, stop=True,
                            )
                            nc.tensor.matmul(
                                ps[:, 512:1024 - off],
                                qkT[64:128, 4 + hp, j * 128:(j + 1) * 128],
                                qkT[64:128, hp, i * 512 + off:(i + 1) * 512],
                                start=True, stop=True,
                            )
                            ee = attn_sb.tile([128, 1024], F32R, tag="E")
                            nc.scalar.activation(
                                ee[:, off:1024 - off], ps[:, off:1024 - off],
                                AF.Exp, scale=0.125)
                            if m >= 0:
                                nc.vector.tensor_mul(
                                    ee[:, off:off + 128],
                                    ee[:, off:off + 128], mask_sb[:])
                                nc.vector.tensor_mul(
                                    ee[:, 512:640],
                                    ee[:, 512:640], mask_sb[:])
                            vrow = v_all[:, j, :].rearrange(
                                "p (h x) -> p h x", x=65)
                            nc.tensor.matmul(
                                oa[:, off:512],
                                vrow[:, 2 * hp, :],
                                ee[:, off:512],
                                start=(j == 0), stop=(j == njt - 1),
                                skip_group_check=True,
                            )
                            nc.tensor.matmul(
                                ob[:, off:512],
                                vrow[:, 2 * hp + 1, :],
                                ee[:, 512:1024 - off],
                                start=(j == 0), stop=(j == njt - 1),
                                skip_group_check=True,
                            )
                        # normalize; y^T overwrites the dead q^T columns
                        for hh, po in ((0, oa), (1, ob)):
                            rc = norm_sb.tile([1, 512], F32, tag="recip")
                            nc.vector.reciprocal(rc[:], po[64:65, :])
                            bc = norm_sb.tile([64, 512], F32, tag="bcast")
                            nc.gpsimd.partition_broadcast(bc[:], rc[:])
                            nc.vector.tensor_mul(
                                qkT[64 * hh:64 * hh + 64,
                                    hp, i * 512:(i + 1) * 512],
                                po[0:64, :], bc[:])

                # projection: out[t, :] = sum_cc yT[:, cc, t]^T @ wo
                for tt in range(NTT):
                    po = pss_pool.tile([128, 1024], F32, tag="ps")
                    for cc in range(4):
                        for nb in range(2):
                            nc.tensor.matmul(
                                po[:, nb * 512:(nb + 1) * 512],
                                qkT[:, cc, tt * 128:(tt + 1) * 128],
                                wo_t[:, cc, nb * 512:(nb + 1) * 512],
                                start=(cc == 0), stop=(cc == 3),
                                skip_group_check=True,
                            )
                    osb = io_pool.tile([128, 1024], F32, tag="osb")
                    if tt % 2 == 0:
                        nc.vector.tensor_copy(osb[:], po[:])
                    else:
                        nc.scalar.copy(osb[:], po[:])
                    nc.sync.dma_start(out_d[tt * 128:(tt + 1) * 128, :], osb[:])

    nc.compile()
    return nc


def _get_nc():
    if "nc" not in _NC_CACHE:
        _NC_CACHE["nc"] = build_kernel()
    return _NC_CACHE["nc"]


def _make_mask() -> np.ndarray:
    p = np.arange(128)[:, None]
    c = np.arange(128)[None, :]
    return (c >= p).astype(np.float32)


def _make_in_maps(inputs):
    """Build the 8 per-core input dicts from the full (unsharded) inputs."""
    x = np.asarray(inputs["x"], dtype=np.float32)
    w_attn = np.asarray(inputs["w_attn"], dtype=np.float32)
    b_attn = np.asarray(inputs["b_attn"], dtype=np.float32)
    w_proj = np.asarray(inputs["w_proj"], dtype=np.float32)
    mask = _make_mask()
    in_maps = []
    for core in range(8):
        b, g = core // 2, core % 2
        xp = np.zeros((T, CIN), dtype=np.float32)
        xp[:, 0:C] = x[b]
        xp[:, C] = 1.0
        cols = np.concatenate([
            np.arange(g * 512, (g + 1) * 512),
            np.arange(C + g * 512, C + (g + 1) * 512),
            np.arange(2 * C + g * 512, 2 * C + (g + 1) * 512),
        ])
        wq = np.zeros((CIN, 1536), dtype=np.float32)
        wq[0:C, :] = w_attn[:, cols]
        wq[C, :] = b_attn[cols]
        wo = np.ascontiguousarray(w_proj[g * 512:(g + 1) * 512, :])
        in_maps.append({"x": xp, "wqkv": wq, "wo": wo, "mask": mask})
    return in_maps


def kernel(x, w_attn, b_attn, w_proj, b_proj):
    b_proj = np.asarray(b_proj, dtype=np.float32)
    in_maps = _make_in_maps({"x": x, "w_attn": w_attn, "b_attn": b_attn,
                             "w_proj": w_proj})
    nc = _get_nc()
    res = bass_utils.run_bass_kernel_spmd(nc, in_maps, core_ids=list(range(8)))
    out = np.empty((B, T, C), dtype=np.float32)
    for b in range(B):
        out[b] = res.results[2 * b]["out"] + res.results[2 * b + 1]["out"] + b_proj
    return out
